# revision 47
# baseline (speedup 1.0000x reference)
"""Trainium2 Bass kernel for nn_Attention_62715112456978.

The reference attention is algebraically rank-1: keys/queries/values are
outer products x ⊗ w, so per batch b

    dot[b,q,k]   = c_b * x[b,q] * x[b,k],   c_b = sum_e wq*wk / sqrt(e)
    softmax-out  = m[b,q] * wv[b,:],        m[b,q] = sum_k A[b,q,k]*x[b,k]
    final        = elu(m[b,q] * r_b + v[b,q]),  r_b = sum_e wv*wo

with wq/wk/wv/wo = |state @ W.T + b| (only the products c, r are needed,
and |a|*|b| = |a*b|, so the abs never has to be materialized).

Engine mapping (per chunk of 128 batches, transposed layout
[(b2,k) partitions, (b', q) free] with b = 2*b' + b2):

  - L = c_b x_q x_k comes straight out of TensorE: one matmul per quarter
    with stationary cxm[b,(b2,k)] = cx[b,k]*par_mask and moving operand
    xsp[b,(b',q)] = x[b,q]*delta(b'=b>>1) (host-staged sparse layout).
  - exp on ScalarE, PSUM -> SBUF bf16.
  - EX = E * x_k on VectorE (x_k is per-partition-constant in this layout,
    broadcast along q).
  - Σ_k (numer/denom) are TensorE matmuls with a block-ones stationary.
  - A single DMA per quarter rearranges the [2, (b',q)] matmul outputs
    back to batch-on-partition layout for the cheap final math.

Sharding: pure data parallel over batch; 8 cores x 512 batches each.
"""

import numpy as np

import concourse.bacc as bacc
import concourse.bass as bass
import concourse.tile as tile
from concourse import mybir
from concourse.bass_utils import run_bass_kernel_spmd

F32 = mybir.dt.float32
BF16 = mybir.dt.bfloat16
I32 = mybir.dt.int32

N_CORES = 8
B_FULL = 4096
BC = B_FULL // N_CORES  # 512 batches per core
CH = 128                # batches per chunk (contraction dim of the L matmul)
NCH = BC // CH          # 4 chunks per core
NQ = 4                  # quarters per chunk (PSUM-bank sized pieces)
QW = 4096 // NQ         # 1024 free elements per quarter = 16 b-pairs x 64 q
T = 64                  # sequence length
D = 128                 # d_state
NW = 5 * 128 - 64       # 576 = wk|wq|wv|wo (128 each) + V (64) output columns
RSQRT_E = float(1.0 / np.sqrt(128.0))
C_SHIFT = 40.0          # global exp shift; cancels in softmax, avoids overflow

_compiled = {}


def _build():
    nc = bacc.Bacc("TRN2", target_bir_lowering=False, debug=False,
                   num_devices=N_CORES)
    xd = nc.dram_tensor("x", [BC, T], F32, kind="ExternalInput")
    std = nc.dram_tensor("stateT", [D, BC], BF16, kind="ExternalInput")
    xkd = nc.dram_tensor("xkt", [BC, T], BF16, kind="ExternalInput")
    xbd = nc.dram_tensor("xbf", [BC, T], BF16, kind="ExternalInput")
    idxd = nc.dram_tensor("scatidx", [CH, NQ * T], mybir.dt.int16,
                          kind="ExternalInput")
    cstd = nc.dram_tensor("consts", [128, 258], F32, kind="ExternalInput")
    wd = nc.dram_tensor("wcatT", [D, NW], BF16, kind="ExternalInput")
    bd = nc.dram_tensor("biasrep", [CH, NW], F32, kind="ExternalInput")
    od = nc.dram_tensor("out", [BC, T], F32, kind="ExternalOutput")

    with tile.TileContext(nc) as tc:
        with (
            tc.tile_pool(name="const", bufs=1) as cpool,
            tc.tile_pool(name="keep", bufs=1) as keep,
            tc.tile_pool(name="io", bufs=4) as iop,
            tc.tile_pool(name="xin", bufs=2) as xinp,
            tc.tile_pool(name="eq", bufs=3) as eqp,
            tc.tile_pool(name="red", bufs=2) as redp,
            tc.tile_pool(name="small", bufs=2) as smp,
        ):
            # one-time constants
            wcat = cpool.tile([D, NW], BF16)
            nc.sync.dma_start(wcat[:], wd[:])
            stT_all = cpool.tile([D, BC], BF16)
            nc.sync.dma_start(stT_all[:], std[:])
            brep = cpool.tile([CH, NW], F32)
            nc.sync.dma_start(brep[:], bd[:])
            shift = cpool.tile([128, 1], F32)
            nc.gpsimd.memset(shift[:], -C_SHIFT)
            cst = cpool.tile([128, 258], F32)
            nc.sync.dma_start(cst[:], cstd[:])
            ident = cst[:, 0:128]
            maskpar = cst[:, 128:256]
            wredf = cst[:, 256:258]
            wred = cpool.tile([128, 2], BF16)
            nc.vector.tensor_copy(wred[:], wredf)
            scatidx = cpool.tile([CH, NQ * T], mybir.dt.int16)
            nc.sync.dma_start(scatidx[:], idxd[:])
            scatidx = cpool.tile([CH, NQ * T], mybir.dt.int16)
            nc.sync.dma_start(scatidx[:], idxd[:])

            # ---------------- phase 1: hypernet + per-chunk prep ----------
            cxm = [None] * NCH
            xkt2 = [None] * NCH
            x_bf = [None] * NCH
            x_bf = [None] * NCH
            r_sb = [None] * NCH
            v_sb = [None] * NCH
            with (
                tc.tile_pool(name="ps1h", bufs=2, space="PSUM") as psh,
            ):
                for ci in range(NCH):
                    bs = ci * CH
                    x_sb = iop.tile([CH, T], F32, tag="x")
                    nc.sync.dma_start(x_sb[:], xd[bs:bs + CH, :])
                    st_sb = iop.tile([CH, D], F32, tag="st")
                    nc.sync.dma_start(st_sb[:], sd[bs:bs + CH, :])

                    stT_ps = psp.tile([D, CH], F32, tag="stT")
                    nc.tensor.transpose(stT_ps[:], st_sb[:], ident[:])
                    stT = smp.tile([D, CH], F32, tag="stTsb")
                    nc.scalar.copy(stT[:], stT_ps[:])

                    hy0 = psh.tile([CH, 512], F32, tag="hy0")
                    nc.tensor.matmul(hy0[:], ones_row[:], biascat[:, 0:512],
                                     start=True, stop=False)
                    nc.tensor.matmul(hy0[:], stT[:], wcat[:, 0:512],
                                     start=False, stop=True)
                    hy1 = psh.tile([CH, T], F32, tag="hy1")
                    nc.tensor.matmul(hy1[:], ones_row[:], biascat[:, 512:NW],
                                     start=True, stop=False)
                    nc.tensor.matmul(hy1[:], stT[:], wcat[:, 512:NW],
                                     start=False, stop=True)

                    hsb = smp.tile([CH, 512], F32, tag="hsb")
                    nc.scalar.copy(hsb[:], hy0[:])
                    v_sb[ci] = keep.tile([CH, T], F32, tag=f"v{ci}", name=f"v{ci}")
                    nc.scalar.copy(v_sb[ci][:], hy1[:])

                    # c = sum_e |wq*wk| / sqrt(e);  r = sum_e |wv*wo|
                    pqk = smp.tile([CH, 128], F32, tag="pqk")
                    nc.vector.tensor_tensor(pqk[:], hsb[:, 0:128],
                                            hsb[:, 128:256],
                                            op=mybir.AluOpType.mult)
                    c0 = smp.tile([CH, 1], F32, tag="c0")
                    nc.vector.tensor_reduce(c0[:], pqk[:],
                                            axis=mybir.AxisListType.X,
                                            op=mybir.AluOpType.add,
                                            apply_absolute_value=True)
                    pvo = smp.tile([CH, 128], F32, tag="pvo")
                    nc.vector.tensor_tensor(pvo[:], hsb[:, 256:384],
                                            hsb[:, 384:512],
                                            op=mybir.AluOpType.mult)
                    r_sb[ci] = keep.tile([CH, 1], F32, tag=f"r{ci}", name=f"r{ci}")
                    nc.vector.tensor_reduce(r_sb[ci][:], pvo[:],
                                            axis=mybir.AxisListType.X,
                                            op=mybir.AluOpType.add,
                                            apply_absolute_value=True)

                    # cxm[b, (b2,k)] = c_b/sqrt(e) * x[b,k] * parity-mask
                    cx = smp.tile([CH, T], F32, tag="cx")
                    nc.vector.tensor_scalar(cx[:], x_sb[:], c0[:], RSQRT_E,
                                            op0=mybir.AluOpType.mult,
                                            op1=mybir.AluOpType.mult)
                    cxdup = smp.tile([CH, 128], F32, tag="cxdup")
                    nc.vector.tensor_copy(cxdup[:, 0:64], cx[:])
                    nc.vector.tensor_copy(cxdup[:, 64:128], cx[:])
                    cxm[ci] = keep.tile([CH, 128], BF16, tag=f"cxm{ci}", name=f"cxm{ci}")
                    nc.vector.tensor_tensor(cxm[ci][:], cxdup[:], maskpar,
                                            op=mybir.AluOpType.mult)

                    # XKT2[(b2,k), b'] = x[2b'+b2, k]  (bf16)
                    xT_ps = psp.tile([T, CH], F32, tag="xT")
                    nc.tensor.transpose(xT_ps[:], x_sb[:], ident[:])
                    xkt2[ci] = keep.tile([CH, T], BF16, tag=f"xk{ci}", name=f"xk{ci}")
                    nc.vector.tensor_copy(xkt2[ci][0:64, :], xT_ps[:, 0:64])
                    nc.vector.tensor_copy(xkt2[ci][64:128, :],
                                          xT_ps[:, 64:128])
                    x_bf[ci] = keep.tile([CH, T], BF16, tag=f"xb{ci}",
                                         name=f"xb{ci}")
                    nc.vector.tensor_copy(x_bf[ci][:], x_sb[:])
                    x_bf[ci] = keep.tile([CH, T], BF16, tag=f"xb{ci}",
                                         name=f"xb{ci}")
                    nc.vector.tensor_copy(x_bf[ci][:], x_sb[:])

            # ---------------- phase 2: the big pipeline -------------------
            # Software-pipelined with a stagger of 2: the reduce matmuls for
            # quarter i are emitted after the L matmul of quarter i+2, so
            # TensorE never stalls waiting for exp (ScalarE) / EX (VectorE)
            # and stays HAM-warm.
            NQT = NCH * NQ
            with (
                tc.tile_pool(name="psL", bufs=2, space="PSUM") as psL,
                tc.tile_pool(name="psR", bufs=2, space="PSUM") as psR,
            ):
                xsp_sb = [None] * NCH
                finals = [None] * NCH
                redsb = [None] * NCH

                def load_chunk(ci):
                    # build the block-diagonal moving operand on-device:
                    # xsp[b, (b & 63)*64 + q] = x[b, q], zeros elsewhere.
                    # One GpSimd local_scatter per PSUM-quarter (it also
                    # zero-fills), per-partition indices select the stripe.
                    if ci >= NCH:
                        return
                    xsp_sb[ci] = xinp.tile([CH, 4096], BF16, tag="xsp",
                                           name=f"xsp{ci}")
                    for q4 in range(NQ):
                        nc.gpsimd.local_scatter(
                            xsp_sb[ci][:, QW * q4:QW * (q4 + 1)],
                            x_bf[ci][:],
                            scatidx[:, T * q4:T * (q4 + 1)],
                            channels=CH, num_elems=QW, num_idxs=T)

                def finish_chunk(ci):
                    bs = ci * CH
                    # rearrange [2,(b2-half,q)] stripes -> [b, q] layout:
                    # 4 col->partition DMAs per chunk (numer/denom x b2)
                    for s, lo in ((0, 0), (1, 32)):
                        for b2 in range(2):
                            srcv = (redsb[ci][lo + b2:lo + b2 + 1, :]
                                    .rearrange("p (bp q) -> p bp q", q=T))
                            dstv = finals[ci][64 * b2:64 * b2 + 64,
                                              s * T:(s + 1) * T]
                            nc.sync.dma_start(dstv, srcv)
                    # z = (numer/denom) * r + v ; out = elu(z)
                    fin = finals[ci]
                    dinv = smp.tile([CH, T], F32, tag="dinv", name="dinv")
                    nc.vector.reciprocal_approx_fast(dinv[:], fin[:, T:2 * T])
                    m_sb = smp.tile([CH, T], F32, tag="m", name="m")
                    nc.vector.tensor_tensor(m_sb[:], fin[:, 0:T], dinv[:],
                                            op=mybir.AluOpType.mult)
                    z2 = smp.tile([CH, T], F32, tag="z2", name="z2")
                    nc.vector.scalar_tensor_tensor(z2[:], m_sb[:],
                                                   r_sb[ci][:], v_sb[ci][:],
                                                   op0=mybir.AluOpType.mult,
                                                   op1=mybir.AluOpType.add)
                    zn = smp.tile([CH, T], F32, tag="zn", name="zn")
                    nc.vector.tensor_scalar(zn[:], z2[:], 0.0, None,
                                            op0=mybir.AluOpType.min)
                    ez = smp.tile([CH, T], F32, tag="ez", name="ez")
                    nc.scalar.activation(ez[:], zn[:],
                                         mybir.ActivationFunctionType.Exp)
                    zp1 = smp.tile([CH, T], F32, tag="zp1", name="zp1")
                    nc.vector.tensor_scalar(zp1[:], z2[:], 0.0, -1.0,
                                            op0=mybir.AluOpType.max,
                                            op1=mybir.AluOpType.add)
                    o_sb = smp.tile([CH, T], F32, tag="o", name="o")
                    nc.vector.tensor_tensor(o_sb[:], zp1[:], ez[:],
                                            op=mybir.AluOpType.add)
                    nc.sync.dma_start(od[bs:bs + CH, :], o_sb[:])

                def reduce_quarter(p):
                    pci, pj, pE, pEX = p
                    cs = pj * QW
                    # numer/denom = sum_k via PE: numer rows {0,1},
                    # denom rows {32,33}
                    red_ps = psR.tile([CH, QW], F32, tag="red", name="red")
                    for h in range(0, QW, 512):
                        nc.tensor.matmul(red_ps[0:2, h:h + 512], wred[:],
                                         pEX[:, h:h + 512],
                                         start=True, stop=True)
                        nc.tensor.matmul(red_ps[32:34, h:h + 512],
                                         wred[:], pE[:, h:h + 512],
                                         start=True, stop=True)
                    # stage PSUM -> SBUF (DMA cannot read PSUM); one
                    # [34, QW] copy covers both stripes at FD cost only.
                    # Alternate engines to balance ScalarE/VectorE load.
                    if (pci * NQ + pj) % 2 == 0:
                        nc.scalar.copy(redsb[pci][0:34, cs:cs + QW],
                                       red_ps[0:34, :])
                    else:
                        nc.vector.tensor_copy(redsb[pci][0:34, cs:cs + QW],
                                              red_ps[0:34, :])
                    if pj == NQ - 1:
                        finish_chunk(pci)
                        load_chunk(pci + 2)

                load_chunk(0)
                load_chunk(1)
                pend = []
                for idx in range(NQT):
                    ci, j = divmod(idx, NQ)
                    if j == 0:
                        finals[ci] = smp.tile([CH, 2 * T], F32, tag="fin",
                                              name=f"fin{ci}")
                        redsb[ci] = redp.tile([64, NQ * QW], F32,
                                              tag="redsb", name=f"redsb{ci}")
                    cs = j * QW
                    # L[(b2,k), (b',q)] = cx[64*b2+b',k] * x[64*b2+b',q]
                    L_ps = psL.tile([CH, QW], F32, tag="L", name="L")
                    for h in range(0, QW, 512):
                        nc.tensor.matmul(L_ps[:, h:h + 512], cxm[ci][:],
                                         xsp_sb[ci][:, cs + h:cs + h + 512],
                                         start=True, stop=True)
                    if len(pend) >= 2:
                        reduce_quarter(pend.pop(0))
                    # E = exp(L - C_SHIFT) -> bf16 SBUF
                    E = eqp.tile([CH, QW], BF16, tag="E", name="E")
                    nc.scalar.activation(E[:], L_ps[:],
                                         mybir.ActivationFunctionType.Exp,
                                         bias=shift[:], scale=1.0)
                    # EX = E * x_k (x_k const along q in this layout)
                    EX = eqp.tile([CH, QW], BF16, tag="EX", name="EX")
                    xkb = (xkt2[ci][:, 16 * j:16 * j + 16]
                           .unsqueeze(2).broadcast_to([CH, 16, T]))
                    Ev = E[:].rearrange("p (bp q) -> p bp q", q=T)
                    nc.vector.tensor_tensor(
                        EX[:].rearrange("p (bp q) -> p bp q", q=T),
                        Ev, xkb, op=mybir.AluOpType.mult)
                    pend.append((ci, j, E, EX))
                for p in pend:
                    reduce_quarter(p)

    nc.compile()
    return nc


def _scatter_idx():
    """Per-partition int16 indices for the 4 local_scatters of a chunk:
    partition b scatters its 64 x-values to columns (b & 63)*64 + q, which
    fall inside PSUM-quarter (b & 63) >> 4; other quarters get -1 (skip)."""
    b = np.arange(CH)
    idx = np.full((CH, NQ * T), -1, dtype=np.int16)
    for q4 in range(NQ):
        sel = (b % 64) // 16 == q4
        rows = b[sel]
        idx[rows, q4 * T:(q4 + 1) * T] = ((rows[:, None] % 64) % 16) * T + \
            np.arange(T)[None, :]
    return idx


def _consts():
    """[ident(128) | maskpar(128) | wredf(2)] fp32 constants."""
    c = np.zeros((128, 258), dtype=np.float32)
    b = np.arange(128)
    c[b, b] = 1.0                                   # identity (transpose)
    c[:, 128:256] = (b[:, None] >> 6) == (b[None, :] >> 6)  # half-split mask
    c[:, 256:258] = (b[:, None] >> 6) == np.arange(2)[None, :]  # reduce W
    return np.ascontiguousarray(c)


def _stage_xk(x):
    """Pre-built bf16 tiles: xkt[chunk-row (b2,k), b'] = x[64*b2+b', k]
    (the per-partition x_k operand in the transposed layout) and a bf16
    copy of x (scatter source)."""
    import ml_dtypes
    xb = x.astype(ml_dtypes.bfloat16)
    g = xb.reshape(-1, 2, 64, T)                  # [chunk, b2, b', k]
    xkt = np.ascontiguousarray(g.transpose(0, 1, 3, 2).reshape(B_FULL, T))
    return xkt, np.ascontiguousarray(xb)


def kernel(**inputs):
    nc = _compiled.get("nc")
    if nc is None:
        nc = _compiled["nc"] = _build()

    x = np.ascontiguousarray(np.asarray(inputs["x"], dtype=np.float32)
                             .reshape(B_FULL, T))
    state = np.ascontiguousarray(np.asarray(inputs["state"], dtype=np.float32))
    scatidx = _scatter_idx()
    consts = _consts()
    xkt, xbf = _stage_xk(x)
    import ml_dtypes
    wcatT = np.ascontiguousarray(np.concatenate(
        [np.asarray(inputs["wk_w"], np.float32).T,
         np.asarray(inputs["wq_w"], np.float32).T,
         np.asarray(inputs["wv_w"], np.float32).T,
         np.asarray(inputs["wo_w"], np.float32).T,
         np.asarray(inputs["V_w"], np.float32).T],
        axis=1).astype(ml_dtypes.bfloat16))
    biasrep = np.ascontiguousarray(np.broadcast_to(np.concatenate(
        [np.asarray(inputs["wk_b"], np.float32),
         np.asarray(inputs["wq_b"], np.float32),
         np.asarray(inputs["wv_b"], np.float32),
         np.asarray(inputs["wo_b"], np.float32),
         np.asarray(inputs["V_b"], np.float32)])[None, :], (CH, NW)).copy())

    in_maps = []
    for i in range(N_CORES):
        sl = slice(i * BC, (i + 1) * BC)
        in_maps.append({
            "x": np.ascontiguousarray(x[sl]),
            "stateT": np.ascontiguousarray(
                state[sl].T.astype(ml_dtypes.bfloat16)),
            "xkt": np.ascontiguousarray(xkt[sl]),
            "xbf": np.ascontiguousarray(xbf[sl]),
            "scatidx": scatidx,
            "consts": consts,
            "wcatT": wcatT,
            "biasrep": biasrep,
        })

    res = run_bass_kernel_spmd(nc, in_maps, core_ids=list(range(N_CORES)))
    out = np.concatenate([res.results[i]["out"] for i in range(N_CORES)],
                         axis=0)
    return out.reshape(B_FULL, 1, T)def _scatter_idx():
    """Per-partition int16 indices for the 4 local_scatters of a chunk:
    partition b scatters its 64 x-values to columns (b & 63)*64 + q, which
    fall inside PSUM-quarter (b & 63) >> 4; other quarters get -1 (skip)."""
    b = np.arange(CH)
    idx = np.full((CH, NQ * T), -1, dtype=np.int16)
    for q4 in range(NQ):
        sel = (b % 64) // 16 == q4
        rows = b[sel]
        idx[rows, q4 * T:(q4 + 1) * T] = ((rows[:, None] % 64) % 16) * T + \
            np.arange(T)[None, :]
    return idx


def kernel(**inputs):
    nc = _compiled.get("nc")
    if nc is None:
        nc = _compiled["nc"] = _build()

    x = np.ascontiguousarray(np.asarray(inputs["x"], dtype=np.float32)
                             .reshape(B_FULL, T))
    state = np.ascontiguousarray(np.asarray(inputs["state"], dtype=np.float32))
    scatidx = _scatter_idx()
    consts = _consts()
    xkt, xbf = _stage_xk(x)
    import ml_dtypes
    wcatT = np.ascontiguousarray(np.concatenate(
        [np.asarray(inputs["wk_w"], np.float32).T,
         np.asarray(inputs["wq_w"], np.float32).T,
         np.asarray(inputs["wv_w"], np.float32).T,
         np.asarray(inputs["wo_w"], np.float32).T,
         np.asarray(inputs["V_w"], np.float32).T],
        axis=1).astype(ml_dtypes.bfloat16))
    biasrep = np.ascontiguousarray(np.broadcast_to(np.concatenate(
        [np.asarray(inputs["wk_b"], np.float32),
         np.asarray(inputs["wq_b"], np.float32),
         np.asarray(inputs["wv_b"], np.float32),
         np.asarray(inputs["wo_b"], np.float32),
         np.asarray(inputs["V_b"], np.float32)])[None, :], (CH, NW)).copy())

    in_maps = []
    for i in range(N_CORES):
        sl = slice(i * BC, (i + 1) * BC)
        in_maps.append({
            "x": np.ascontiguousarray(x[sl]),
            "stateT": np.ascontiguousarray(
                state[sl].T.astype(ml_dtypes.bfloat16)),
            "xkt": np.ascontiguousarray(xkt[sl]),
            "xbf": np.ascontiguousarray(xbf[sl]),
            "scatidx": scatidx,
            "consts": consts,
            "wcatT": wcatT,
            "biasrep": biasrep,
        })

    res = run_bass_kernel_spmd(nc, in_maps, core_ids=list(range(N_CORES)))
    out = np.concatenate([res.results[i]["out"] for i in range(N_CORES)],
                         axis=0)
    return out.reshape(B_FULL, 1, T)


# revision 48
# speedup vs baseline: 1.0109x; 1.0109x over previous
"""Trainium2 Bass kernel for nn_Attention_62715112456978.

The reference attention is algebraically rank-1: keys/queries/values are
outer products x ⊗ w, so per batch b

    dot[b,q,k]   = c_b * x[b,q] * x[b,k],   c_b = sum_e wq*wk / sqrt(e)
    softmax-out  = m[b,q] * wv[b,:],        m[b,q] = sum_k A[b,q,k]*x[b,k]
    final        = elu(m[b,q] * r_b + v[b,q]),  r_b = sum_e wv*wo

with wq/wk/wv/wo = |state @ W.T + b| (only the products c, r are needed,
and |a|*|b| = |a*b|, so the abs never has to be materialized).

Engine mapping (per chunk of 128 batches, transposed layout
[(b2,k) partitions, (b', q) free] with b = 2*b' + b2):

  - L = c_b x_q x_k comes straight out of TensorE: one matmul per quarter
    with stationary cxm[b,(b2,k)] = cx[b,k]*par_mask and moving operand
    xsp[b,(b',q)] = x[b,q]*delta(b'=b>>1) (host-staged sparse layout).
  - exp on ScalarE, PSUM -> SBUF bf16.
  - EX = E * x_k on VectorE (x_k is per-partition-constant in this layout,
    broadcast along q).
  - Σ_k (numer/denom) are TensorE matmuls with a block-ones stationary.
  - A single DMA per quarter rearranges the [2, (b',q)] matmul outputs
    back to batch-on-partition layout for the cheap final math.

Sharding: pure data parallel over batch; 8 cores x 512 batches each.
"""

import numpy as np

import concourse.bacc as bacc
import concourse.bass as bass
import concourse.tile as tile
from concourse import mybir
from concourse.bass_utils import run_bass_kernel_spmd

F32 = mybir.dt.float32
BF16 = mybir.dt.bfloat16
I32 = mybir.dt.int32

N_CORES = 8
B_FULL = 4096
BC = B_FULL // N_CORES  # 512 batches per core
CH = 128                # batches per chunk (contraction dim of the L matmul)
NCH = BC // CH          # 4 chunks per core
NQ = 4                  # quarters per chunk (PSUM-bank sized pieces)
QW = 4096 // NQ         # 1024 free elements per quarter = 16 b-pairs x 64 q
T = 64                  # sequence length
D = 128                 # d_state
NW = 5 * 128 - 64       # 576 = wk|wq|wv|wo (128 each) + V (64) output columns
RSQRT_E = float(1.0 / np.sqrt(128.0))
C_SHIFT = 40.0          # global exp shift; cancels in softmax, avoids overflow

_compiled = {}


def _build():
    nc = bacc.Bacc("TRN2", target_bir_lowering=False, debug=False,
                   num_devices=N_CORES)
    xd = nc.dram_tensor("x", [BC, T], F32, kind="ExternalInput")
    std = nc.dram_tensor("stateT", [D, BC], BF16, kind="ExternalInput")
    xkd = nc.dram_tensor("xkt", [BC, T], BF16, kind="ExternalInput")
    xbd = nc.dram_tensor("xbf", [BC, T], BF16, kind="ExternalInput")
    idxd = nc.dram_tensor("scatidx", [CH, NQ * T], mybir.dt.int16,
                          kind="ExternalInput")
    cstd = nc.dram_tensor("consts", [128, 258], F32, kind="ExternalInput")
    wd = nc.dram_tensor("wcatT", [D, NW], BF16, kind="ExternalInput")
    bd = nc.dram_tensor("biasrep", [CH, NW], F32, kind="ExternalInput")
    od = nc.dram_tensor("out", [BC, T], F32, kind="ExternalOutput")

    with tile.TileContext(nc) as tc:
        with (
            tc.tile_pool(name="const", bufs=1) as cpool,
            tc.tile_pool(name="keep", bufs=1) as keep,
            tc.tile_pool(name="io", bufs=4) as iop,
            tc.tile_pool(name="xin", bufs=2) as xinp,
            tc.tile_pool(name="eq", bufs=3) as eqp,
            tc.tile_pool(name="red", bufs=2) as redp,
            tc.tile_pool(name="small", bufs=2) as smp,
        ):
            # one-time constants
            wcat = cpool.tile([D, NW], BF16)
            nc.sync.dma_start(wcat[:], wd[:])
            stT_all = cpool.tile([D, BC], BF16)
            nc.sync.dma_start(stT_all[:], std[:])
            brep = cpool.tile([CH, NW], F32)
            nc.sync.dma_start(brep[:], bd[:])
            shift = cpool.tile([128, 1], F32)
            nc.gpsimd.memset(shift[:], -C_SHIFT)
            cst = cpool.tile([128, 258], F32)
            nc.sync.dma_start(cst[:], cstd[:])
            ident = cst[:, 0:128]
            maskpar = cst[:, 128:256]
            wredf = cst[:, 256:258]
            wred = cpool.tile([128, 2], BF16)
            nc.vector.tensor_copy(wred[:], wredf)
            scatidx = cpool.tile([CH, NQ * T], mybir.dt.int16)
            nc.sync.dma_start(scatidx[:], idxd[:])
            scatidx = cpool.tile([CH, NQ * T], mybir.dt.int16)
            nc.sync.dma_start(scatidx[:], idxd[:])

            # ---------------- phase 1: hypernet + per-chunk prep ----------
            cxm = [None] * NCH
            xkt2 = [None] * NCH
            x_bf = [None] * NCH
            x_bf = [None] * NCH
            r_sb = [None] * NCH
            v_sb = [None] * NCH
            with (
                tc.tile_pool(name="ps1h", bufs=2, space="PSUM") as psh,
            ):
                for ci in range(NCH):
                    bs = ci * CH
                    x_sb = iop.tile([CH, T], F32, tag="x")
                    nc.sync.dma_start(x_sb[:], xd[bs:bs + CH, :])
                    st_sb = iop.tile([CH, D], F32, tag="st")
                    nc.sync.dma_start(st_sb[:], sd[bs:bs + CH, :])

                    stT_ps = psp.tile([D, CH], F32, tag="stT")
                    nc.tensor.transpose(stT_ps[:], st_sb[:], ident[:])
                    stT = smp.tile([D, CH], F32, tag="stTsb")
                    nc.scalar.copy(stT[:], stT_ps[:])

                    hy0 = psh.tile([CH, 512], F32, tag="hy0")
                    nc.tensor.matmul(hy0[:], ones_row[:], biascat[:, 0:512],
                                     start=True, stop=False)
                    nc.tensor.matmul(hy0[:], stT[:], wcat[:, 0:512],
                                     start=False, stop=True)
                    hy1 = psh.tile([CH, T], F32, tag="hy1")
                    nc.tensor.matmul(hy1[:], ones_row[:], biascat[:, 512:NW],
                                     start=True, stop=False)
                    nc.tensor.matmul(hy1[:], stT[:], wcat[:, 512:NW],
                                     start=False, stop=True)

                    hsb = smp.tile([CH, 512], F32, tag="hsb")
                    nc.scalar.copy(hsb[:], hy0[:])
                    v_sb[ci] = keep.tile([CH, T], F32, tag=f"v{ci}", name=f"v{ci}")
                    nc.scalar.copy(v_sb[ci][:], hy1[:])

                    # c = sum_e |wq*wk| / sqrt(e);  r = sum_e |wv*wo|
                    pqk = smp.tile([CH, 128], F32, tag="pqk")
                    nc.vector.tensor_tensor(pqk[:], hsb[:, 0:128],
                                            hsb[:, 128:256],
                                            op=mybir.AluOpType.mult)
                    c0 = smp.tile([CH, 1], F32, tag="c0")
                    nc.vector.tensor_reduce(c0[:], pqk[:],
                                            axis=mybir.AxisListType.X,
                                            op=mybir.AluOpType.add,
                                            apply_absolute_value=True)
                    pvo = smp.tile([CH, 128], F32, tag="pvo")
                    nc.vector.tensor_tensor(pvo[:], hsb[:, 256:384],
                                            hsb[:, 384:512],
                                            op=mybir.AluOpType.mult)
                    r_sb[ci] = keep.tile([CH, 1], F32, tag=f"r{ci}", name=f"r{ci}")
                    nc.vector.tensor_reduce(r_sb[ci][:], pvo[:],
                                            axis=mybir.AxisListType.X,
                                            op=mybir.AluOpType.add,
                                            apply_absolute_value=True)

                    # cxm[b, (b2,k)] = c_b/sqrt(e) * x[b,k] * parity-mask
                    cx = smp.tile([CH, T], F32, tag="cx")
                    nc.vector.tensor_scalar(cx[:], x_sb[:], c0[:], RSQRT_E,
                                            op0=mybir.AluOpType.mult,
                                            op1=mybir.AluOpType.mult)
                    cxdup = smp.tile([CH, 128], F32, tag="cxdup")
                    nc.vector.tensor_copy(cxdup[:, 0:64], cx[:])
                    nc.vector.tensor_copy(cxdup[:, 64:128], cx[:])
                    cxm[ci] = keep.tile([CH, 128], BF16, tag=f"cxm{ci}", name=f"cxm{ci}")
                    nc.vector.tensor_tensor(cxm[ci][:], cxdup[:], maskpar,
                                            op=mybir.AluOpType.mult)

                    # XKT2[(b2,k), b'] = x[2b'+b2, k]  (bf16)
                    xT_ps = psp.tile([T, CH], F32, tag="xT")
                    nc.tensor.transpose(xT_ps[:], x_sb[:], ident[:])
                    xkt2[ci] = keep.tile([CH, T], BF16, tag=f"xk{ci}", name=f"xk{ci}")
                    nc.vector.tensor_copy(xkt2[ci][0:64, :], xT_ps[:, 0:64])
                    nc.vector.tensor_copy(xkt2[ci][64:128, :],
                                          xT_ps[:, 64:128])
                    x_bf[ci] = keep.tile([CH, T], BF16, tag=f"xb{ci}",
                                         name=f"xb{ci}")
                    nc.vector.tensor_copy(x_bf[ci][:], x_sb[:])
                    x_bf[ci] = keep.tile([CH, T], BF16, tag=f"xb{ci}",
                                         name=f"xb{ci}")
                    nc.vector.tensor_copy(x_bf[ci][:], x_sb[:])

            # ---------------- phase 2: the big pipeline -------------------
            # Software-pipelined with a stagger of 2: the reduce matmuls for
            # quarter i are emitted after the L matmul of quarter i+2, so
            # TensorE never stalls waiting for exp (ScalarE) / EX (VectorE)
            # and stays HAM-warm.
            NQT = NCH * NQ
            with (
                tc.tile_pool(name="psL", bufs=2, space="PSUM") as psL,
                tc.tile_pool(name="psR", bufs=2, space="PSUM") as psR,
            ):
                xsp_sb = [None] * NCH
                finals = [None] * NCH
                redsb = [None] * NCH

                def load_chunk(ci):
                    # build the block-diagonal moving operand on-device:
                    # xsp[b, (b & 63)*64 + q] = x[b, q], zeros elsewhere.
                    # One GpSimd local_scatter per PSUM-quarter (it also
                    # zero-fills), per-partition indices select the stripe.
                    if ci >= NCH:
                        return
                    xsp_sb[ci] = xinp.tile([CH, 4096], BF16, tag="xsp",
                                           name=f"xsp{ci}")
                    for q4 in range(NQ):
                        nc.gpsimd.local_scatter(
                            xsp_sb[ci][:, QW * q4:QW * (q4 + 1)],
                            x_bf[ci][:],
                            scatidx[:, T * q4:T * (q4 + 1)],
                            channels=CH, num_elems=QW, num_idxs=T)

                def finish_chunk(ci):
                    bs = ci * CH
                    # rearrange [2,(b2-half,q)] stripes -> [b, q] layout:
                    # 4 col->partition DMAs per chunk (numer/denom x b2)
                    # last chunk: ScalarE is idle in the tail, so split
                    # the rearranges across both DMA queues there.
                    last = ci == NCH - 1
                    for s, lo in ((0, 0), (1, 32)):
                        for b2 in range(2):
                            srcv = (redsb[ci][lo + b2:lo + b2 + 1, :]
                                    .rearrange("p (bp q) -> p bp q", q=T))
                            dstv = finals[ci][64 * b2:64 * b2 + 64,
                                              s * T:(s + 1) * T]
                            eng = nc.scalar if (last and b2 == 1) else nc.sync
                            eng.dma_start(dstv, srcv)
                    # z = (numer/denom) * r + v ; out = elu(z)
                    fin = finals[ci]
                    dinv = smp.tile([CH, T], F32, tag="dinv", name="dinv")
                    nc.vector.reciprocal_approx_fast(dinv[:], fin[:, T:2 * T])
                    m_sb = smp.tile([CH, T], F32, tag="m", name="m")
                    nc.vector.tensor_tensor(m_sb[:], fin[:, 0:T], dinv[:],
                                            op=mybir.AluOpType.mult)
                    z2 = smp.tile([CH, T], F32, tag="z2", name="z2")
                    nc.vector.scalar_tensor_tensor(z2[:], m_sb[:],
                                                   r_sb[ci][:], v_sb[ci][:],
                                                   op0=mybir.AluOpType.mult,
                                                   op1=mybir.AluOpType.add)
                    zn = smp.tile([CH, T], F32, tag="zn", name="zn")
                    nc.vector.tensor_scalar(zn[:], z2[:], 0.0, None,
                                            op0=mybir.AluOpType.min)
                    ez = smp.tile([CH, T], F32, tag="ez", name="ez")
                    nc.scalar.activation(ez[:], zn[:],
                                         mybir.ActivationFunctionType.Exp)
                    zp1 = smp.tile([CH, T], F32, tag="zp1", name="zp1")
                    nc.vector.tensor_scalar(zp1[:], z2[:], 0.0, -1.0,
                                            op0=mybir.AluOpType.max,
                                            op1=mybir.AluOpType.add)
                    o_sb = smp.tile([CH, T], F32, tag="o", name="o")
                    nc.vector.tensor_tensor(o_sb[:], zp1[:], ez[:],
                                            op=mybir.AluOpType.add)
                    nc.sync.dma_start(od[bs:bs + CH, :], o_sb[:])

                def reduce_quarter(p):
                    pci, pj, pE, pEX = p
                    cs = pj * QW
                    # numer/denom = sum_k via PE: numer rows {0,1},
                    # denom rows {32,33}
                    red_ps = psR.tile([CH, QW], F32, tag="red", name="red")
                    for h in range(0, QW, 512):
                        nc.tensor.matmul(red_ps[0:2, h:h + 512], wred[:],
                                         pEX[:, h:h + 512],
                                         start=True, stop=True)
                        nc.tensor.matmul(red_ps[32:34, h:h + 512],
                                         wred[:], pE[:, h:h + 512],
                                         start=True, stop=True)
                    # stage PSUM -> SBUF (DMA cannot read PSUM); one
                    # [34, QW] copy covers both stripes at FD cost only.
                    # Alternate engines to balance ScalarE/VectorE load.
                    if (pci * NQ + pj) % 2 == 0:
                        nc.scalar.copy(redsb[pci][0:34, cs:cs + QW],
                                       red_ps[0:34, :])
                    else:
                        nc.vector.tensor_copy(redsb[pci][0:34, cs:cs + QW],
                                              red_ps[0:34, :])
                    if pj == NQ - 1:
                        finish_chunk(pci)
                        load_chunk(pci + 2)

                load_chunk(0)
                load_chunk(1)
                pend = []
                for idx in range(NQT):
                    ci, j = divmod(idx, NQ)
                    if j == 0:
                        finals[ci] = smp.tile([CH, 2 * T], F32, tag="fin",
                                              name=f"fin{ci}")
                        redsb[ci] = redp.tile([64, NQ * QW], F32,
                                              tag="redsb", name=f"redsb{ci}")
                    cs = j * QW
                    # L[(b2,k), (b',q)] = cx[64*b2+b',k] * x[64*b2+b',q]
                    L_ps = psL.tile([CH, QW], F32, tag="L", name="L")
                    for h in range(0, QW, 512):
                        nc.tensor.matmul(L_ps[:, h:h + 512], cxm[ci][:],
                                         xsp_sb[ci][:, cs + h:cs + h + 512],
                                         start=True, stop=True)
                    if len(pend) >= 2:
                        reduce_quarter(pend.pop(0))
                    # E = exp(L - C_SHIFT) -> bf16 SBUF
                    E = eqp.tile([CH, QW], BF16, tag="E", name="E")
                    nc.scalar.activation(E[:], L_ps[:],
                                         mybir.ActivationFunctionType.Exp,
                                         bias=shift[:], scale=1.0)
                    # EX = E * x_k (x_k const along q in this layout)
                    EX = eqp.tile([CH, QW], BF16, tag="EX", name="EX")
                    xkb = (xkt2[ci][:, 16 * j:16 * j + 16]
                           .unsqueeze(2).broadcast_to([CH, 16, T]))
                    Ev = E[:].rearrange("p (bp q) -> p bp q", q=T)
                    nc.vector.tensor_tensor(
                        EX[:].rearrange("p (bp q) -> p bp q", q=T),
                        Ev, xkb, op=mybir.AluOpType.mult)
                    pend.append((ci, j, E, EX))
                for p in pend:
                    reduce_quarter(p)

    nc.compile()
    return nc


def _scatter_idx():
    """Per-partition int16 indices for the 4 local_scatters of a chunk:
    partition b scatters its 64 x-values to columns (b & 63)*64 + q, which
    fall inside PSUM-quarter (b & 63) >> 4; other quarters get -1 (skip)."""
    b = np.arange(CH)
    idx = np.full((CH, NQ * T), -1, dtype=np.int16)
    for q4 in range(NQ):
        sel = (b % 64) // 16 == q4
        rows = b[sel]
        idx[rows, q4 * T:(q4 + 1) * T] = ((rows[:, None] % 64) % 16) * T + \
            np.arange(T)[None, :]
    return idx


def _consts():
    """[ident(128) | maskpar(128) | wredf(2)] fp32 constants."""
    c = np.zeros((128, 258), dtype=np.float32)
    b = np.arange(128)
    c[b, b] = 1.0                                   # identity (transpose)
    c[:, 128:256] = (b[:, None] >> 6) == (b[None, :] >> 6)  # half-split mask
    c[:, 256:258] = (b[:, None] >> 6) == np.arange(2)[None, :]  # reduce W
    return np.ascontiguousarray(c)


def _stage_xk(x):
    """Pre-built bf16 tiles: xkt[chunk-row (b2,k), b'] = x[64*b2+b', k]
    (the per-partition x_k operand in the transposed layout) and a bf16
    copy of x (scatter source)."""
    import ml_dtypes
    xb = x.astype(ml_dtypes.bfloat16)
    g = xb.reshape(-1, 2, 64, T)                  # [chunk, b2, b', k]
    xkt = np.ascontiguousarray(g.transpose(0, 1, 3, 2).reshape(B_FULL, T))
    return xkt, np.ascontiguousarray(xb)


def kernel(**inputs):
    nc = _compiled.get("nc")
    if nc is None:
        nc = _compiled["nc"] = _build()

    x = np.ascontiguousarray(np.asarray(inputs["x"], dtype=np.float32)
                             .reshape(B_FULL, T))
    state = np.ascontiguousarray(np.asarray(inputs["state"], dtype=np.float32))
    scatidx = _scatter_idx()
    consts = _consts()
    xkt, xbf = _stage_xk(x)
    import ml_dtypes
    wcatT = np.ascontiguousarray(np.concatenate(
        [np.asarray(inputs["wk_w"], np.float32).T,
         np.asarray(inputs["wq_w"], np.float32).T,
         np.asarray(inputs["wv_w"], np.float32).T,
         np.asarray(inputs["wo_w"], np.float32).T,
         np.asarray(inputs["V_w"], np.float32).T],
        axis=1).astype(ml_dtypes.bfloat16))
    biasrep = np.ascontiguousarray(np.broadcast_to(np.concatenate(
        [np.asarray(inputs["wk_b"], np.float32),
         np.asarray(inputs["wq_b"], np.float32),
         np.asarray(inputs["wv_b"], np.float32),
         np.asarray(inputs["wo_b"], np.float32),
         np.asarray(inputs["V_b"], np.float32)])[None, :], (CH, NW)).copy())

    in_maps = []
    for i in range(N_CORES):
        sl = slice(i * BC, (i + 1) * BC)
        in_maps.append({
            "x": np.ascontiguousarray(x[sl]),
            "stateT": np.ascontiguousarray(
                state[sl].T.astype(ml_dtypes.bfloat16)),
            "xkt": np.ascontiguousarray(xkt[sl]),
            "xbf": np.ascontiguousarray(xbf[sl]),
            "scatidx": scatidx,
            "consts": consts,
            "wcatT": wcatT,
            "biasrep": biasrep,
        })

    res = run_bass_kernel_spmd(nc, in_maps, core_ids=list(range(N_CORES)))
    out = np.concatenate([res.results[i]["out"] for i in range(N_CORES)],
                         axis=0)
    return out.reshape(B_FULL, 1, T)def _scatter_idx():
    """Per-partition int16 indices for the 4 local_scatters of a chunk:
    partition b scatters its 64 x-values to columns (b & 63)*64 + q, which
    fall inside PSUM-quarter (b & 63) >> 4; other quarters get -1 (skip)."""
    b = np.arange(CH)
    idx = np.full((CH, NQ * T), -1, dtype=np.int16)
    for q4 in range(NQ):
        sel = (b % 64) // 16 == q4
        rows = b[sel]
        idx[rows, q4 * T:(q4 + 1) * T] = ((rows[:, None] % 64) % 16) * T + \
            np.arange(T)[None, :]
    return idx


def kernel(**inputs):
    nc = _compiled.get("nc")
    if nc is None:
        nc = _compiled["nc"] = _build()

    x = np.ascontiguousarray(np.asarray(inputs["x"], dtype=np.float32)
                             .reshape(B_FULL, T))
    state = np.ascontiguousarray(np.asarray(inputs["state"], dtype=np.float32))
    scatidx = _scatter_idx()
    consts = _consts()
    xkt, xbf = _stage_xk(x)
    import ml_dtypes
    wcatT = np.ascontiguousarray(np.concatenate(
        [np.asarray(inputs["wk_w"], np.float32).T,
         np.asarray(inputs["wq_w"], np.float32).T,
         np.asarray(inputs["wv_w"], np.float32).T,
         np.asarray(inputs["wo_w"], np.float32).T,
         np.asarray(inputs["V_w"], np.float32).T],
        axis=1).astype(ml_dtypes.bfloat16))
    biasrep = np.ascontiguousarray(np.broadcast_to(np.concatenate(
        [np.asarray(inputs["wk_b"], np.float32),
         np.asarray(inputs["wq_b"], np.float32),
         np.asarray(inputs["wv_b"], np.float32),
         np.asarray(inputs["wo_b"], np.float32),
         np.asarray(inputs["V_b"], np.float32)])[None, :], (CH, NW)).copy())

    in_maps = []
    for i in range(N_CORES):
        sl = slice(i * BC, (i + 1) * BC)
        in_maps.append({
            "x": np.ascontiguousarray(x[sl]),
            "stateT": np.ascontiguousarray(
                state[sl].T.astype(ml_dtypes.bfloat16)),
            "xkt": np.ascontiguousarray(xkt[sl]),
            "xbf": np.ascontiguousarray(xbf[sl]),
            "scatidx": scatidx,
            "consts": consts,
            "wcatT": wcatT,
            "biasrep": biasrep,
        })

    res = run_bass_kernel_spmd(nc, in_maps, core_ids=list(range(N_CORES)))
    out = np.concatenate([res.results[i]["out"] for i in range(N_CORES)],
                         axis=0)
    return out.reshape(B_FULL, 1, T)
